# revision 1
# baseline (speedup 1.0000x reference)
import numpy as np
import concourse.bass as bass
import concourse.bacc as bacc_mod
import concourse.mybir as mybir
from concourse import tile
from concourse.bass_utils import run_bass_kernel_spmd

B, I, K, O, D = 128, 1152, 8, 32, 16
NC = 8
IL = I // NC          # 144 capsules per core
OD = O * D            # 512
CH = 4                # i-chunk size
NCH = IL // CH        # 24 chunks
EPS = 1e-8
NROUTES = 3

F32 = mybir.dt.float32
BF16 = mybir.dt.bfloat16
ADD = mybir.AluOpType.add
MULT = mybir.AluOpType.mult
AF = mybir.ActivationFunctionType
AX = mybir.AxisListType


def _build():
    nc = bacc_mod.Bacc()
    wc_d = nc.declare_dram_parameter("wc", [K, IL, B + OD], BF16,
                                     isOutput=False)
    id_d = nc.declare_dram_parameter("ident", [B, B], BF16, isOutput=False)
    v_d = nc.declare_dram_parameter("vout", [B, OD], F32, isOutput=True)
    # collective bounce buffers (unique per route: avoids DMA reuse waits)
    ar_in = [nc.dram_tensor(f"ar_in{r}", [B, OD], F32) for r in range(3)]
    ar_out = [nc.dram_tensor(f"ar_out{r}", [B, OD], F32) for r in range(3)]

    with tile.TileContext(nc) as tc:
        with (
            tc.tile_pool(name="big", bufs=1) as big,
            tc.tile_pool(name="ld", bufs=2) as ld,
            tc.tile_pool(name="work", bufs=2) as work,
            tc.tile_pool(name="small", bufs=1) as small,
            tc.tile_pool(name="ps_a", bufs=4, space="PSUM") as ps_a,
            tc.tile_pool(name="ps_z", bufs=2, space="PSUM") as ps_z,
            tc.tile_pool(name="ps_s", bufs=1, space="PSUM") as ps_s,
        ):
            # persistent tiles
            xh = big.tile([B, IL * OD], BF16, tag="xh")      # 147KB/part
            ident = small.tile([B, B], BF16, tag="id")
            nc.sync.dma_start(out=ident[:], in_=id_d[:])
            zc = small.tile([B, 1], F32, tag="zc")
            nc.vector.memset(zc[:], 0.0)
            nc.const_aps.aps[(F32, 0.0)] = zc[:]
            zbuf = big.tile([B, IL * O], F32, tag="z")        # 18KB/part (z then e)
            cbuf = big.tile([B, IL * O], BF16, tag="c")       # 9KB/part
            u16 = small.tile([B, OD], BF16, tag="u16")
            vsum = small.tile([B, OD], F32, tag="vsum")       # v1+v2 accumulator
            sar = small.tile([B, OD], F32, tag="sar")         # allreduced s

            # ---------- phase A: x_hat + route-1 s (uniform c) ----------
            s_ps = ps_s.tile([B, OD], F32, tag="sps")
            BOD = B + OD
            for ic in range(NCH):
                w_t = ld.tile([K, CH * BOD], BF16, tag="wt")
                nc.gpsimd.dma_start(
                    out=w_t[:], in_=wc_d[:, ic * CH:(ic + 1) * CH, :])
                for j in range(CH):
                    i_g = ic * CH + j
                    xh_ps = ps_a.tile([B, OD], F32, tag="xhps")
                    nc.tensor.matmul(
                        xh_ps[:], w_t[:, j * BOD:j * BOD + B],
                        w_t[:, j * BOD + B:(j + 1) * BOD],
                        start=True, stop=True)
                    # evacuate to bf16 slab, alternate DVE/ACT
                    dst = xh[:, i_g * OD:(i_g + 1) * OD]
                    # DVE:ACT ~ 3:2 split matches their PSUM-copy rates
                    if i_g % 5 < 3:
                        nc.vector.tensor_copy(dst, xh_ps[:])
                    else:
                        nc.scalar.copy(dst, xh_ps[:])
                    # route-1 s accumulation: s1 = sum_i x_hat_i (uniform c)
                    nc.tensor.matmul(
                        s_ps[:], ident[:], dst,
                        start=(i_g == 0), stop=(i_g == IL - 1))

            def all_reduce_s(s_psum, scale, rno):
                s_loc = work.tile([B, OD], F32, tag="sq_t2")
                nc.scalar.mul(s_loc[:], s_psum[:], scale)
                nc.sync.dma_start(out=ar_in[rno][:], in_=s_loc[:])
                nc.gpsimd.collective_compute(
                    "AllReduce", ADD,
                    replica_groups=[list(range(NC))],
                    ins=[ar_in[rno][:]], outs=[ar_out[rno][:]])
                sar = small.tile([B, OD], F32, tag="sarX")
                nc.sync.dma_start(out=sar[:], in_=ar_out[rno][:])
                return sar

            def squash_to(vdst32, sar, roundno):
                # sar holds s [B, (o,d)]; compute v = s * q/((1+q)sqrt(q+eps))
                s3 = sar[:].rearrange("p (o d) -> p o d", o=O)
                t = work.tile([B, OD], F32, tag="sq_t2")
                nc.vector.tensor_mul(t[:], sar[:], sar[:])
                q = small.tile([B, O], F32, tag="qsq")
                nc.vector.tensor_reduce(
                    q[:], t[:].rearrange("p (o d) -> p o d", o=O),
                    axis=AX.X, op=ADD)
                qe = small.tile([B, O], F32, tag="qesq")
                nc.vector.tensor_scalar_add(qe[:], q[:], EPS)
                r = small.tile([B, O], F32, tag="rsq")
                nc.scalar.activation(r[:], qe[:], AF.Sqrt)
                t1 = small.tile([B, O], F32, tag="t1sq")
                nc.vector.scalar_tensor_tensor(
                    t1[:], q[:], 1.0, r[:], op0=ADD, op1=MULT)
                t2 = small.tile([B, O], F32, tag="t2sq")
                nc.vector.reciprocal(t2[:], t1[:])
                f = small.tile([B, O], F32, tag="fsq")
                nc.vector.tensor_mul(f[:], q[:], t2[:])
                fb = f[:].broadcast_to((B, O, D))
                nc.vector.tensor_mul(
                    vdst32[:].rearrange("p (o d) -> p o d", o=O), s3, fb)

            # ---------- route 1 ----------
            sar1 = all_reduce_s(s_ps, 1.0 / O, 0)
            squash_to(vsum, sar1, 1)
            nc.vector.tensor_copy(u16[:], vsum[:])   # u2 = v1 (bf16)

            # ---------- routes 2..3 ----------
            for rt in range(2, NROUTES + 1):
                last = rt == NROUTES
                # z = sum_d xhat * u  (DVE mult -> PE accum over d)
                ub = u16[:].rearrange("p (x o d) -> p x o d", x=1, o=O) \
                           .broadcast_to((B, CH, O, D))
                for ic in range(NCH):
                    y = work.tile([B, CH * OD], BF16, tag="y")
                    xs = xh[:, ic * CH * OD:(ic + 1) * CH * OD] \
                        .rearrange("p (i o d) -> p i o d", o=O, d=D)
                    nc.vector.tensor_mul(
                        y[:].rearrange("p (i o d) -> p i o d", o=O, d=D),
                        xs, ub)
                    z_ps = ps_z.tile([B, CH * O], F32, tag="zps")
                    yv = y[:].rearrange("p (i o d) -> p i o d", o=O, d=D)
                    for d in range(D):
                        nc.tensor.matmul(
                            z_ps[:], ident[:], yv[:, :, :, d],
                            start=(d == 0), stop=(d == D - 1))
                    # exp straight out of PSUM -> e (fp32, zbuf slab)
                    nc.scalar.activation(
                        zbuf[:, ic * CH * O:(ic + 1) * CH * O], z_ps[:],
                        AF.Exp)
                # softmax denom over o, then c = e * (1/sigma)
                sig = small.tile([B, IL], F32, tag="sig")
                nc.vector.tensor_reduce(
                    sig[:], zbuf[:].rearrange("p (i o) -> p i o", o=O),
                    axis=AX.X, op=ADD)
                rho = small.tile([B, IL], F32, tag="rho")
                nc.vector.reciprocal(rho[:], sig[:])
                rb = rho[:].rearrange("p (i x) -> p i x", x=1).broadcast_to((B, IL, O))
                nc.vector.tensor_mul(
                    cbuf[:].rearrange("p (i o) -> p i o", o=O),
                    zbuf[:].rearrange("p (i o) -> p i o", o=O), rb)
                # s = sum_i c * xhat  (DVE mult -> PE accum over i)
                s_ps2 = ps_s.tile([B, OD], F32, tag="sps")
                for ic in range(NCH):
                    sy = work.tile([B, CH * OD], BF16, tag="y")
                    cb = cbuf[:, ic * CH * O:(ic + 1) * CH * O] \
                        .rearrange("p (i o x) -> p i o x", o=O, x=1) \
                        .broadcast_to((B, CH, O, D))
                    xs = xh[:, ic * CH * OD:(ic + 1) * CH * OD] \
                        .rearrange("p (i o d) -> p i o d", o=O, d=D)
                    nc.vector.tensor_mul(
                        sy[:].rearrange("p (i o d) -> p i o d", o=O, d=D),
                        xs, cb)
                    for j in range(CH):
                        i_g = ic * CH + j
                        nc.tensor.matmul(
                            s_ps2[:], ident[:],
                            sy[:, j * OD:(j + 1) * OD],
                            start=(i_g == 0), stop=(i_g == IL - 1))
                sarR = all_reduce_s(s_ps2, 1.0, rt - 1)
                if last:
                    vout_t = small.tile([B, OD], F32, tag="vfin")
                    squash_to(vout_t, sarR, rt)
                    nc.sync.dma_start(out=v_d[:], in_=vout_t[:])
                else:
                    v2 = small.tile([B, OD], F32, tag="vfin")
                    squash_to(v2, sarR, rt)
                    nc.vector.tensor_add(vsum[:], vsum[:], v2[:])
                    nc.vector.tensor_copy(u16[:], vsum[:])  # u3 = v1+v2
    nc.compile()
    return nc


def _filter_bir(bir_json: bytes) -> bytes:
    """Drop same-ring WAW waits on DMAs (ring FIFO makes them redundant);
    the DIRECT2D descriptor only holds one wait command."""
    import json
    d = json.loads(bir_json)
    for fn in d.get("functions", []):
        for blk in fn.get("blocks", []):
            for inst in blk.get("instructions", []):
                if inst.get("opcode") != "DMACopy":
                    continue
                si = inst.get("sync_info") or {}
                waits = si.get("on_wait") or []
                if len(waits) <= 1:
                    continue
                ups = {u.get("ant_name") for u in (si.get("on_update") or [])}
                kept = [w for w in waits if w.get("ant_name") not in ups]
                if len(kept) < len(waits):
                    si["on_wait"] = kept
    return json.dumps(d).encode()


def _install_bir_filter():
    from concourse import bass2jax, bass_utils

    orig = bass_utils.compile_bir_kernel

    def patched(bir_json, tmpdir, neff_name="file.neff"):
        return orig(_filter_bir(bir_json), tmpdir, neff_name)

    bass2jax.compile_bir_kernel = patched


def _make_in_maps(x: np.ndarray, W: np.ndarray):
    ident = np.eye(B, dtype=np.float32)
    in_maps = []
    for c in range(NC):
        sl = slice(c * IL, (c + 1) * IL)
        xt = np.ascontiguousarray(
            x[:, sl, :].transpose(2, 1, 0)).astype(np.float32)  # [K, IL, B]
        wk = np.ascontiguousarray(
            W[sl].transpose(2, 0, 1, 3).reshape(K, IL, OD)).astype(np.float32)
        wc = np.concatenate([xt, wk], axis=2)  # [K, IL, B+OD]
        in_maps.append({"wc": _bf16(wc), "ident": _bf16(ident)})
    return in_maps


def kernel(x: np.ndarray, W: np.ndarray) -> np.ndarray:
    _install_bir_filter()
    nc = _build()
    in_maps = _make_in_maps(x, W)
    res = run_bass_kernel_spmd(nc, in_maps, list(range(NC)))
    v = np.asarray(res.results[0]["vout"], dtype=np.float32)
    return v.reshape(B, O, D)


def _bf16(a: np.ndarray):
    import jax.numpy as jnp
    return np.asarray(jnp.asarray(a, dtype=jnp.bfloat16))


if __name__ == "__main__":
    nc = _build()
    print("IR build OK")

